# revision 90
# baseline (speedup 1.0000x reference)
"""Trainium2 Bass kernel for DiffusionConv (Chebyshev graph diffusion).

Math (reference):
    x0 = [feat; feat]                       # [2N, T*F]
    x1 = A @ x0                             # A sparse: A[dst, src] = sum ef
    x2 = 2*A@x1 - x0 ; x3 = 2*A@x2 - x1
    out = concat([feat, x1[:N], x1[N:], x2[:N], x2[N:], x3[:N], x3[N:]]) @ W + b

Strategy (8 NeuronCores, SPMD single program):
  - Edges sorted by dst; dst-range sharded 8-way (6250 slots/core).
  - Per core, edges packed into tiles of <=128 distinct dst slots; within a
    tile edges are split by source half (fwd: src<N, bwd: src>=N) so the Q7
    dma_gather op can use int16 row indices into per-half tables.
  - Per (tile, half): 2 dma_gathers (512 idx each, on distinct SWDGE queues)
    pull source rows (T*F values each) into SBUF; a one-hot scatter matrix S
    (precomputed on host, with ef pre-doubled for the Chebyshev 2*A) folds the
    per-edge multiply and the segment-sum into TensorE matmuls in PSUM.
  - Diffusion state tables are PADDED per-tile: tile t of core c occupies rows
    [c*TPC + t*128, +128) (TPC = ntiles*128).  The per-tile result write is
    then a plain compile-time-offset DMA on the Sync engine (HWDGE) - no
    GPSIMD scatter - so next-tile gathers overlap current-tile matmuls.
  - Chebyshev combine is ONE DVE op per tile: k=0: y = 0.5*psum (S holds 2ef);
    k>=1: y = psum - xp (xp = x_{k-2} tile: k=1 a plain load from the
    host-prepped padded x0 table, k=2 the SBUF-resident x1 tile).
  - Gathers pass exact runtime edge counts via num_idxs_reg (index tails are
    -1 = skipped); padding costs no SWDGE descriptor-generation time, which is
    the step-phase pace (~2.2ns/descriptor, shared across the 4 queues).
  - Per step one AllGather of the padded slice (collective outputs must be
    contiguous; chunk-major layouts that would allow overlapped chunked
    collectives overflow the int16 gather-index limit).  Warmup collectives at
    t=0 pay the CC setup under step 0.  x3: pairwise exchange only; the
    x3-independent final gathers are pre-emitted BEFORE its trigger so the
    pair-skew rendezvous wait is filled with useful work.
  - Final linear: 1024-row dma_gathers (node-major), TensorE transpose to
    feature-major, block-diagonal weight matmuls (N=512) accumulate
    out^T per t-pair in PSUM; bias via ACT engine; host reassembles.

The full (unsharded) inputs come in; host-side numpy does index/layout
preprocessing only (sorting, tiling, padding) - all FLOPs of the module
run on the NeuronCores.
"""

import os
import sys

sys.path.insert(0, "/opt/trn_rl_repo")

import numpy as np

import concourse.bacc as bacc
import concourse.bass as bass
import concourse.mybir as mybir
import concourse.tile as tile

# ---------------------------------------------------------------- problem dims
N = int(os.environ.get("DIFF_N", 25000))
T = 8
F = 32
OUTF = 64
STEPS = 3
ROW = T * F            # 256 values per node-row
TWO_N = 2 * N
CORES = 8
SPC = TWO_N // CORES   # dst slots per core
CPF = int(os.environ.get("DIFF_CPF", 9))  # chunks (of 128 edges) per tile side
CAP = CPF * 128        # edge capacity per tile side
# each side is gathered in two ops (dma_gather faults above 1024 indices):
HCA = (CPF - CPF // 2) * 128  # first-half capacity
HCB = (CPF // 2) * 128        # second-half capacity
NFIN = N // CORES      # final-linear rows (n) per core
NFCH = (NFIN + 127) // 128  # final-linear chunks
NQ = 4                 # SWDGE queues (Q7 cpu pairs) for dma_gather
NF4 = (NFIN + 1023) // 1024  # final-linear 1024-node macro-chunks

FP32 = mybir.dt.float32
BF16 = mybir.dt.bfloat16
I32 = mybir.dt.int32
I16 = mybir.dt.int16

# table / gather dtype: float32 (precise) or bfloat16 (half the DMA traffic)
TAB_DT = BF16 if os.environ.get("DIFF_TAB_BF16", "1") == "1" else FP32
TAB_NP = np.dtype("bfloat16") if TAB_DT is BF16 else np.dtype("float32")


# ================================================================ host prep
def _prep(feat, ef, W, b, src, dst):
    """Build per-core tiled edge metadata. Returns (in_maps, ntiles)."""
    feat = np.ascontiguousarray(np.asarray(feat), dtype=np.float32).reshape(N, ROW)
    ef = np.asarray(ef, dtype=np.float32)
    src = np.asarray(src, dtype=np.int64)
    dst = np.asarray(dst, dtype=np.int64)

    order = np.argsort(dst, kind="stable")
    s_src = src[order]
    s_dst = dst[order]
    s_ef = ef[order]

    core_edge_bounds = np.searchsorted(s_dst, np.arange(CORES + 1) * SPC)

    # ---- per-core greedy tiling (capacity per source half)
    per_core = []
    for c in range(CORES):
        lo, hi = core_edge_bounds[c], core_edge_bounds[c + 1]
        cs, cd, ce = s_src[lo:hi], s_dst[lo:hi] - c * SPC, s_ef[lo:hi]
        fwd_mask = cs < N
        counts_f = np.bincount(cd[fwd_mask], minlength=SPC)
        counts_b = np.bincount(cd[~fwd_mask], minlength=SPC)
        starts = np.concatenate([[0], np.cumsum(counts_f + counts_b)])
        tiles = []  # (slot_lo, slot_hi, edge_lo, edge_hi)
        slot = 0
        while slot < SPC:
            t_lo = slot
            nf = nb = 0
            while (
                slot < SPC
                and slot - t_lo < 128
                and nf + counts_f[slot] <= CAP
                and nb + counts_b[slot] <= CAP
            ):
                nf += counts_f[slot]
                nb += counts_b[slot]
                slot += 1
            if slot == t_lo:
                raise ValueError("node degree exceeds tile capacity")
            tiles.append((t_lo, slot, starts[t_lo], starts[slot]))
        per_core.append((cs, cd, ce, tiles))

    ntiles = max(len(p[3]) for p in per_core)
    TPC = ntiles * 128  # padded table rows per core

    # slot (local, per core) -> padded table position (local, per core)
    pos_local = np.zeros((CORES, SPC), np.int64)
    for c in range(CORES):
        tiles = per_core[c][3]
        for t, (sl, sh, _, _) in enumerate(tiles):
            pos_local[c, sl:sh] = t * 128 + np.arange(sh - sl)

    def padded_pos(glob):
        """global slot id in [0, 2N) -> row in padded [8*TPC] table."""
        cc = glob // SPC
        return cc * TPC + pos_local[cc, glob % SPC]

    # block-diagonal final weights: for block bi, half h: [128, 128] with
    # rows p = tq*32+f, cols m = (tq'-2h)*64+o, val = W[bi*32+f, o] iff tq==tq'
    Wf = np.asarray(W, np.float32)
    wblk = np.zeros((128, 14, 128), np.float32)
    for bi in range(7):
        for h in range(2):
            for tq in (2 * h, 2 * h + 1):
                wblk[
                    tq * 32 : (tq + 1) * 32,
                    bi * 2 + h,
                    (tq - 2 * h) * 64 : (tq - 2 * h + 1) * 64,
                ] = Wf[bi * 32 : (bi + 1) * 32, :]
    bias2 = np.tile(np.asarray(b, np.float32), 2).reshape(128, 1)

    def wrap16(vals):
        """int64 idx array (multiple of 16) -> [128, len//16] int16 wrapped."""
        return np.tile(vals.astype(np.int16).reshape(-1, 16).T, (8, 1))

    in_maps = []
    for c in range(CORES):
        cs, cd, ce, tiles = per_core[c]
        gidx0 = np.full((128, ntiles, 2, CAP // 16), -1, np.int16)
        gidx12 = np.full((128, ntiles, 2, CAP // 16), -1, np.int16)
        # null tiles: 16 valid row-0 reads per half, rest skipped (-1)
        gidx0[:, :, :, 0] = 0
        gidx0[:, :, :, HCA // 16] = 0
        gidx12[:, :, :, 0] = 0
        gidx12[:, :, :, HCA // 16] = 0
        cnts = np.full((ntiles, 2, 2), 16, np.int32)
        smat = np.zeros((ntiles, 128, 2 * CPF, 128), np.float32)
        for t, (sl, sh, el, eh) in enumerate(tiles):
            e_src = cs[el:eh]
            e_slot = cd[el:eh] - sl
            e_w = ce[el:eh]
            for side in (0, 1):
                m = (e_src < N) if side == 0 else (e_src >= N)
                srcs = e_src[m]
                s_slot = e_slot[m]
                s_w = e_w[m]
                # k=0 table: feat rows (src mod N); k>=1: padded table pos
                real0 = (srcs - side * N).astype(np.int64)
                real12 = padded_pos(srcs) - side * 4 * TPC
                n = len(real0)
                # exact runtime counts per 512-half: [real | <=16 zeros | -1s]
                # (-1 tail is skipped by the gather ucode; the runtime count
                # must equal the number of non-negative indices)
                mask = -np.ones(CAP, np.int64)  # -1 = skip, 0 = valid row 0
                mask[:n] = 0
                n0 = min(n, HCA)
                keep0 = min(16, HCA - n0)
                mask[n0 : n0 + keep0] = 0
                n1 = n - n0
                keep1 = min(16, HCB - n1)
                mask[HCA + n1 : HCA + n1 + keep1] = 0
                cnts[t, side, 0] = n0 + keep0
                cnts[t, side, 1] = n1 + keep1
                idx0 = np.where(mask < 0, -1, 0)
                idx12 = idx0.copy()
                idx0[:n] = real0
                idx12[:n] = real12
                s_slot = np.concatenate([s_slot, np.zeros(CAP - n, np.int64)])
                s_w = np.concatenate([s_w, np.zeros(CAP - n, np.float32)])
                # edge g -> (partition g%128, chunk g//128); idx wrapped by 16
                gidx0[:, t, side, :] = wrap16(idx0)
                gidx12[:, t, side, :] = wrap16(idx12)
                # S[p, cc, slot] = 2*ef for chunk cc = side*CPF + g//128
                # (Chebyshev 2*A folded in; k=0 output rescaled by 0.5)
                e_p = np.arange(CAP) % 128
                e_cc = side * CPF + np.arange(CAP) // 128
                smat[t, e_p, e_cc, s_slot] += 2.0 * s_w

        # x0 slice in padded-tile layout (xp source for step k=1)
        x0pad = np.zeros((TPC, ROW), np.float32)
        x0pad[pos_local[c]] = feat[(c * SPC + np.arange(SPC)) % N]

        # final-linear gather indices, per (block, 1024-node macro-chunk)
        if c < 4:
            nbase = c * SPC
        else:
            nbase = (c - 4) * SPC + NFIN
        fidx = np.zeros((128, 7, NF4, 1024 // 16), np.int16)
        fcnt = np.zeros(NF4, np.int32)
        for ch in range(NF4):
            nreal = min(1024, NFIN - ch * 1024)
            fkeep = min(16, 1024 - nreal)
            fcnt[ch] = nreal + fkeep
            nl = ch * 1024 + np.arange(1024)
            ng = nbase + np.minimum(nl, NFIN - 1)
            cfw = ng // SPC          # core owning fwd half (0..3)
            loc = ng - (c % 4) * SPC
            i_fwd = cfw * TPC + pos_local[cfw, ng % SPC]       # x12 fwd pos
            i_bwd = cfw * TPC + pos_local[cfw + 4, ng % SPC]   # x12 bwd (-4TPC)
            i_3f = pos_local[c % 4, loc]                 # x3 fwd (group slot 0)
            i_3b = pos_local[c % 4 + 4, loc]             # x3 bwd (group slot 1)
            # blocks: feat, x1f, x1b, x2f, x2b, x3f, x3b
            colmap = [ng, i_fwd, i_bwd, i_fwd, i_bwd, i_3f, i_3b]
            for bi in range(7):
                vals = np.asarray(colmap[bi], np.int64).copy()
                vals[nreal : nreal + fkeep] = 0
                vals[nreal + fkeep :] = -1
                fidx[:, bi, ch, :] = wrap16(vals)
        in_maps.append(
            {
                "feat": feat.astype(TAB_NP, copy=True),
                "gidx0": gidx0.reshape(128, ntiles * 2 * (CAP // 16)).copy(),
                "gidx12": gidx12.reshape(128, ntiles * 2 * (CAP // 16)).copy(),
                "smat": smat.reshape(ntiles * 128, 2 * CPF * 128).astype(TAB_NP),
                "x0pad": x0pad.astype(TAB_NP),
                "fidx": fidx.reshape(128, 7 * NF4 * 64).copy(),
                "gcnt": np.concatenate(
                    [cnts.reshape(-1), fcnt]
                ).reshape(1, ntiles * 4 + NF4),
                "wblk": wblk.reshape(128, 14 * 128).astype(np.dtype("bfloat16")),
                "bias2": bias2.copy(),
                "ident": np.eye(128, dtype=np.dtype("bfloat16")),
            }
        )
    return in_maps, ntiles


# ================================================================ bass program
def _build(ntiles):
    nc = bacc.Bacc(
        "TRN2",
        target_bir_lowering=False,
        debug=False,
        num_devices=CORES,
        num_swdge_queues=NQ,
    )

    TPC = ntiles * 128

    feat_t = nc.dram_tensor("feat", [N, ROW], TAB_DT, kind="ExternalInput")
    gidx0_t = nc.dram_tensor(
        "gidx0", [128, ntiles * 2 * (CAP // 16)], I16, kind="ExternalInput"
    )
    gidx12_t = nc.dram_tensor(
        "gidx12", [128, ntiles * 2 * (CAP // 16)], I16, kind="ExternalInput"
    )
    smat_t = nc.dram_tensor(
        "smat", [ntiles * 128, 2 * CPF * 128], TAB_DT, kind="ExternalInput"
    )
    x0pad_t = nc.dram_tensor("x0pad", [TPC, ROW], TAB_DT, kind="ExternalInput")
    fidx_t = nc.dram_tensor(
        "fidx", [128, 7 * NF4 * 64], I16, kind="ExternalInput"
    )
    wblk_t = nc.dram_tensor("wblk", [128, 14 * 128], BF16, kind="ExternalInput")
    bias2_t = nc.dram_tensor("bias2", [128, 1], FP32, kind="ExternalInput")
    ident_t = nc.dram_tensor("ident", [128, 128], BF16, kind="ExternalInput")
    gcnt_t = nc.dram_tensor(
        "gcnt", [1, ntiles * 4 + NF4], I32, kind="ExternalInput"
    )

    outT = nc.dram_tensor("outT", [T, OUTF, NFIN], FP32, kind="ExternalOutput")

    # internal DRAM: per-step slice tensors (padded tile rows)
    slice_bufs = [
        nc.dram_tensor(f"slice{k}", [TPC, ROW], TAB_DT) for k in range(STEPS)
    ]
    x1t = nc.dram_tensor("x1t", [CORES * TPC, ROW], TAB_DT, addr_space="Shared")
    x2t = nc.dram_tensor("x2t", [CORES * TPC, ROW], TAB_DT, addr_space="Shared")
    x3t = nc.dram_tensor("x3t", [2 * TPC, ROW], TAB_DT)

    rg_all = [list(range(CORES))]
    rg_pair = [[c, c + 4] for c in range(4)]

    warm_in = nc.dram_tensor("warm_in", [16, ROW], TAB_DT)
    warm_out = nc.dram_tensor("warm_out", [128, ROW], TAB_DT, addr_space="Shared")
    warm_out2 = nc.dram_tensor("warm_out2", [32, ROW], TAB_DT)

    ICW = CAP // 16  # idx columns per (tile, half)

    with tile.TileContext(nc, num_cores=CORES) as tc:
        with (
            tc.tile_pool(name="const", bufs=1) as constp,
            tc.tile_pool(name="meta", bufs=1) as metap,
        ):
            # step-0 prerequisites first so its gathers start ASAP; the
            # final-phase constants load behind them on the sync queue
            gcnt_s = metap.tile([1, ntiles * 4 + NF4], I32)
            nc.sync.dma_start(gcnt_s[:], gcnt_t[:])
            gidx0_s = metap.tile([128, ntiles * 2 * ICW], I16)
            nc.sync.dma_start(gidx0_s[:], gidx0_t[:])
            gidx12_s = metap.tile([128, ntiles * 2 * ICW], I16)
            nc.sync.dma_start(gidx12_s[:], gidx12_t[:])
            wblk_s = constp.tile([128, 14 * 128], BF16)
            nc.sync.dma_start(wblk_s[:], wblk_t[:])
            bias2_s = constp.tile([128, 1], FP32)
            nc.sync.dma_start(bias2_s[:], bias2_t[:])
            ident_s = constp.tile([128, 128], BF16)
            nc.sync.dma_start(ident_s[:], ident_t[:])
            fidx_s = metap.tile([128, 7 * NF4 * 64], I16)
            nc.sync.dma_start(fidx_s[:], fidx_t[:])
            cregs = [
                nc.alloc_register(mybir.EngineType.Pool, f"gcnt{i}")
                for i in range(4)
            ]

            # ---------------- diffusion steps
            with (
                tc.tile_pool(name="big", bufs=1) as bigp,
                tc.tile_pool(name="gat", bufs=3) as gatp,
                tc.tile_pool(name="sml", bufs=3) as smlp,
                tc.tile_pool(name="ys", bufs=3) as ysp,
                tc.tile_pool(name="xpp", bufs=3) as xpp,
                tc.tile_pool(name="ps", bufs=4, space="PSUM") as psp,
                tc.tile_pool(name="fin", bufs=2) as finp,
                tc.tile_pool(name="fps", bufs=1, space="PSUM") as fpsp,
            ):
                # tiny warmup collectives: pay CC setup/rendezvous under step 0
                nc.gpsimd.collective_compute(
                    "AllGather",
                    mybir.AluOpType.bypass,
                    replica_groups=rg_all,
                    ins=[warm_in[:].opt()],
                    outs=[warm_out[:].opt()],
                )
                nc.gpsimd.collective_compute(
                    "AllGather",
                    mybir.AluOpType.bypass,
                    replica_groups=rg_pair,
                    ins=[warm_in[:].opt()],
                    outs=[warm_out2[:].opt()],
                )

                # final-linear gather source blocks (x3 halves appended below)
                blocks = [
                    feat_t[0:N, :],
                    x1t[0 : 4 * TPC, :],
                    x1t[4 * TPC : 8 * TPC, :],
                    x2t[0 : 4 * TPC, :],
                    x2t[4 * TPC : 8 * TPC, :],
                    x3t[0:TPC, :],
                    x3t[TPC : 2 * TPC, :],
                ]
                pre_gf = {}

                y0_all = bigp.tile([128, ntiles * ROW], TAB_DT, name="y0")
                for k in range(STEPS):
                    if k == 0:
                        halves = [feat_t[0:N, :], feat_t[0:N, :]]
                        gsrc = gidx0_s
                    elif k == 1:
                        halves = [x1t[0 : 4 * TPC, :], x1t[4 * TPC : 8 * TPC, :]]
                        gsrc = gidx12_s
                    else:
                        halves = [x2t[0 : 4 * TPC, :], x2t[4 * TPC : 8 * TPC, :]]
                        gsrc = gidx12_s
                    for t in range(ntiles):
                        psum = psp.tile([128, ROW], FP32, tag="acc")
                        Ssb = smlp.tile([128, 2 * CPF * 128], TAB_DT, tag="S")
                        nc.sync.dma_start(Ssb[:], smat_t[t * 128 : (t + 1) * 128, :])
                        if k == 1:
                            xp = xpp.tile([128, ROW], TAB_DT, tag="xp")
                            nc.sync.dma_start(
                                xp[:], x0pad_t[t * 128 : (t + 1) * 128, :]
                            )
                        for side in (0, 1):
                            G = gatp.tile(
                                [128, CPF * ROW], TAB_DT, tag=f"G{side}",
                                name=f"G{side}",
                            )
                            if k == 0 and t < 3:
                                # first use of each rotating buffer: clear
                                # uninitialized SBUF (skipped gather rows are
                                # multiplied by S=0; 0*NaN would poison psum)
                                nc.vector.memset(G[:], 0.0)
                            icb = (t * 2 + side) * ICW
                            hcA = CPF - CPF // 2  # chunks in first half-gather
                            for hh in range(2):
                                ri = side * 2 + hh
                                nc.gpsimd.reg_load(
                                    cregs[ri],
                                    gcnt_s[0:1, (t * 2 + side) * 2 + hh :
                                           (t * 2 + side) * 2 + hh + 1],
                                )
                                glo = 0 if hh == 0 else hcA * ROW
                                ghi = hcA * ROW if hh == 0 else CPF * ROW
                                ilo = 0 if hh == 0 else HCA // 16
                                ihi = HCA // 16 if hh == 0 else ICW
                                nc.gpsimd.dma_gather(
                                    G[:, glo:ghi].rearrange(
                                        "p (c r) -> p c r",
                                        c=(hcA if hh == 0 else CPF - hcA),
                                    ),
                                    halves[side],
                                    gsrc[:, icb + ilo : icb + ihi],
                                    HCA if hh == 0 else HCB,
                                    cregs[ri],
                                    ROW,
                                    queue_num=side * 2 + hh,
                                )
                            for j in range(CPF):
                                cc = side * CPF + j
                                nc.tensor.matmul(
                                    out=psum[:],
                                    lhsT=Ssb[:, cc * 128 : (cc + 1) * 128],
                                    rhs=G[:, j * ROW : (j + 1) * ROW],
                                    start=(side == 0 and j == 0),
                                    stop=(side == 1 and j == CPF - 1),
                                )
                        # psum holds 2*A@x ; Chebyshev combine in ONE DVE op
                        if k == 0:
                            y_sl = y0_all[:, t * ROW : (t + 1) * ROW]
                            nc.vector.tensor_scalar(
                                out=y_sl,
                                in0=psum[:],
                                scalar1=0.5,
                                scalar2=None,
                                op0=mybir.AluOpType.mult,
                            )
                        else:
                            y_t = ysp.tile([128, ROW], TAB_DT, tag="y")
                            y_sl = y_t[:]
                            xp_sl = (
                                xp[:]
                                if k == 1
                                else y0_all[:, t * ROW : (t + 1) * ROW]
                            )
                            nc.vector.tensor_tensor(
                                out=y_sl,
                                in0=psum[:],
                                in1=xp_sl,
                                op=mybir.AluOpType.subtract,
                            )
                        # plain write into this tile's padded rows (Sync HWDGE)
                        nc.sync.dma_start(
                            slice_bufs[k][t * 128 : (t + 1) * 128, :],
                            y_sl,
                        )
                    if k == STEPS - 1:
                        # pre-emit x3-independent final gathers BEFORE the x3
                        # collective trigger: the trigger's rendezvous wait
                        # (pair-core skew) otherwise stalls the GPSIMD queue
                        for ch4 in range(2):
                            for bi in range(5):
                                Gf = finp.tile(
                                    [128, 8 * ROW], TAB_DT, tag=f"Gf{bi}",
                                    name=f"Gf{bi}",
                                )
                                pre_gf[(ch4, bi)] = Gf
                                ri = (ch4 * 5 + bi) % NQ
                                nc.gpsimd.reg_load(
                                    cregs[ri],
                                    gcnt_s[
                                        0:1, ntiles * 4 + ch4 : ntiles * 4 + ch4 + 1
                                    ],
                                )
                                nc.gpsimd.dma_gather(
                                    Gf[:].rearrange("p (c r) -> p c r", c=8),
                                    blocks[bi],
                                    fidx_s[
                                        :,
                                        (bi * NF4 + ch4) * 64 : (bi * NF4 + ch4 + 1)
                                        * 64,
                                    ],
                                    1024,
                                    cregs[ri],
                                    ROW,
                                    queue_num=ri,
                                )
                    nc.gpsimd.collective_compute(
                        "AllGather",
                        mybir.AluOpType.bypass,
                        replica_groups=rg_all if k < STEPS - 1 else rg_pair,
                        ins=[slice_bufs[k][:].opt()],
                        outs=[((x1t, x2t, x3t)[k])[:].opt()],
                    )

                # ---------------- final linear: node-major 1024-row gathers,
                # TensorE transpose to feature-major, block-diagonal weight
                # matmuls (no narrow regroup copies)
                for ch4 in range(NF4):
                    Gfs = []
                    for bi, tab in enumerate(blocks):
                        if (ch4, bi) in pre_gf:
                            Gfs.append(pre_gf[(ch4, bi)])
                            continue
                        Gf = finp.tile(
                            [128, 8 * ROW], TAB_DT, tag=f"Gf{bi}", name=f"Gf{bi}"
                        )
                        Gfs.append(Gf)
                        ri = (ch4 * 7 + bi) % NQ
                        nc.gpsimd.reg_load(
                            cregs[ri],
                            gcnt_s[0:1, ntiles * 4 + ch4 : ntiles * 4 + ch4 + 1],
                        )
                        nc.gpsimd.dma_gather(
                            Gf[:].rearrange("p (c r) -> p c r", c=8),
                            tab,
                            fidx_s[:, (bi * NF4 + ch4) * 64 : (bi * NF4 + ch4 + 1) * 64],
                            1024,
                            cregs[ri],
                            ROW,
                            queue_num=ri,
                        )
                    for half4 in range(2):  # 512-node halves of the macro-chunk
                        p0 = ch4 * 1024 + half4 * 512
                        if p0 >= NFIN:
                            break
                        ncols = min(512, NFIN - p0)
                        for hh in range(2):  # (t,f) granule: t in [4hh, 4hh+4)
                            hT = []
                            for bi in range(7):
                                hTb = finp.tile(
                                    [128, 512], BF16, tag=f"hT{bi}", name=f"hT{bi}"
                                )
                                hT.append(hTb[:])
                                for s4 in range(4):
                                    if p0 + s4 * 128 >= NFIN:
                                        break
                                    sub = half4 * 4 + s4
                                    ptr = fpsp.tile(
                                        [128, 128], BF16, tag=f"ptr{bi % 2}",
                                        name="ptr",
                                    )
                                    nc.tensor.transpose(
                                        out=ptr[:],
                                        in_=Gfs[bi][
                                            :, sub * ROW + hh * 128 : sub * ROW
                                            + (hh + 1) * 128
                                        ],
                                        identity=ident_s[:],
                                    )
                                    nc.vector.tensor_copy(
                                        hTb[:, s4 * 128 : (s4 + 1) * 128], ptr[:]
                                    )
                            for h2 in range(2):  # t pair within granule
                                pso = fpsp.tile(
                                    [128, 512], FP32, tag=f"pso{h2}", name="pso"
                                )
                                for bi in range(7):
                                    nc.tensor.matmul(
                                        out=pso[:],
                                        lhsT=wblk_s[
                                            :,
                                            (bi * 2 + h2) * 128 : (bi * 2 + h2 + 1)
                                            * 128,
                                        ],
                                        rhs=hT[bi],
                                        start=(bi == 0),
                                        stop=(bi == 6),
                                    )
                                stage = finp.tile(
                                    [128, 512], FP32, tag=f"st{h2}", name="st"
                                )
                                nc.scalar.activation(
                                    out=stage[:],
                                    in_=pso[:],
                                    func=mybir.ActivationFunctionType.Identity,
                                    bias=bias2_s[:],
                                    scale=1.0,
                                )
                                # rows m = (t - (4hh+2h2))*64 + o -> outT[t, o]
                                st = stage[:]
                                out_ap = bass.AP(
                                    outT,
                                    ((hh * 4 + 2 * h2) * OUTF) * NFIN + p0,
                                    [[NFIN, 128], [1, ncols]],
                                )
                                st_ap = bass.AP(
                                    st.tensor, st.offset, [st.ap[0], [1, ncols]]
                                )
                                nc.sync.dma_start(out=out_ap, in_=st_ap)

    nc.compile()
    return nc


_NC_CACHE = {}


def _get_nc(ntiles):
    if ntiles not in _NC_CACHE:
        _NC_CACHE[ntiles] = _build(ntiles)
    return _NC_CACHE[ntiles]


# ================================================================ entry point
def kernel(feat, ef, W, b, src, dst):
    from concourse.bass_utils import run_bass_kernel_spmd

    in_maps, ntiles = _prep(feat, ef, W, b, src, dst)
    nc = _get_nc(ntiles)
    res = run_bass_kernel_spmd(nc, in_maps, core_ids=list(range(CORES)))
    out = np.zeros((N, T, OUTF), np.float32)
    for c in range(CORES):
        o = np.asarray(res.results[c]["outT"], np.float32).reshape(T, OUTF, NFIN)
        if c < 4:
            nbase = c * SPC
        else:
            nbase = (c - 4) * SPC + NFIN
        out[nbase : nbase + NFIN] = o.transpose(2, 0, 1)
    return out


# revision 91
# speedup vs baseline: 1.0537x; 1.0537x over previous
"""Trainium2 Bass kernel for DiffusionConv (Chebyshev graph diffusion).

Math (reference):
    x0 = [feat; feat]                       # [2N, T*F]
    x1 = A @ x0                             # A sparse: A[dst, src] = sum ef
    x2 = 2*A@x1 - x0 ; x3 = 2*A@x2 - x1
    out = concat([feat, x1[:N], x1[N:], x2[:N], x2[N:], x3[:N], x3[N:]]) @ W + b

Strategy (8 NeuronCores, SPMD single program):
  - Edges sorted by dst; dst-range sharded 8-way (6250 slots/core).
  - Per core, edges packed into tiles of <=128 distinct dst slots; within a
    tile edges are split by source half (fwd: src<N, bwd: src>=N) so the Q7
    dma_gather op can use int16 row indices into per-half tables.
  - Per (tile, half): 2 dma_gathers (512 idx each, on distinct SWDGE queues)
    pull source rows (T*F values each) into SBUF; a one-hot scatter matrix S
    (precomputed on host, with ef pre-doubled for the Chebyshev 2*A) folds the
    per-edge multiply and the segment-sum into TensorE matmuls in PSUM.
  - Diffusion state tables are PADDED per-tile: tile t of core c occupies rows
    [c*TPC + t*128, +128) (TPC = ntiles*128).  The per-tile result write is
    then a plain compile-time-offset DMA on the Sync engine (HWDGE) - no
    GPSIMD scatter - so next-tile gathers overlap current-tile matmuls.
  - Chebyshev combine is ONE DVE op per tile: k=0: y = 0.5*psum (S holds 2ef);
    k>=1: y = psum - xp (xp = x_{k-2} tile: k=1 a plain load from the
    host-prepped padded x0 table, k=2 the SBUF-resident x1 tile).
  - Gathers pass exact runtime edge counts via num_idxs_reg (index tails are
    -1 = skipped); padding costs no SWDGE descriptor-generation time, which is
    the step-phase pace (~2.2ns/descriptor, shared across the 4 queues).
  - Per step one AllGather of the padded slice (collective outputs must be
    contiguous; chunk-major layouts that would allow overlapped chunked
    collectives overflow the int16 gather-index limit).  Warmup collectives at
    t=0 pay the CC setup under step 0.  x3: pairwise exchange only; the
    x3-independent final gathers are pre-emitted BEFORE its trigger so the
    pair-skew rendezvous wait is filled with useful work.
  - Final linear: 1024-row dma_gathers (node-major), TensorE transpose to
    feature-major, block-diagonal weight matmuls (N=512) accumulate
    out^T per t-pair in PSUM; bias via ACT engine; host reassembles.

The full (unsharded) inputs come in; host-side numpy does index/layout
preprocessing only (sorting, tiling, padding) - all FLOPs of the module
run on the NeuronCores.
"""

import os
import sys

sys.path.insert(0, "/opt/trn_rl_repo")

import numpy as np

import concourse.bacc as bacc
import concourse.bass as bass
import concourse.mybir as mybir
import concourse.tile as tile

# ---------------------------------------------------------------- problem dims
N = int(os.environ.get("DIFF_N", 25000))
T = 8
F = 32
OUTF = 64
STEPS = 3
ROW = T * F            # 256 values per node-row
TWO_N = 2 * N
CORES = 8
SPC = TWO_N // CORES   # dst slots per core
CPF = int(os.environ.get("DIFF_CPF", 9))  # chunks (of 128 edges) per tile side
CAP = CPF * 128        # edge capacity per tile side
# each side is gathered in two ops (dma_gather faults above 1024 indices):
HCA = (CPF - CPF // 2) * 128  # first-half capacity
HCB = (CPF // 2) * 128        # second-half capacity
NFIN = N // CORES      # final-linear rows (n) per core
NFCH = (NFIN + 127) // 128  # final-linear chunks
NQ = 4                 # SWDGE queues (Q7 cpu pairs) for dma_gather
NF4 = (NFIN + 1023) // 1024  # final-linear 1024-node macro-chunks

FP32 = mybir.dt.float32
BF16 = mybir.dt.bfloat16
I32 = mybir.dt.int32
I16 = mybir.dt.int16

# table / gather dtype: float32 (precise) or bfloat16 (half the DMA traffic)
TAB_DT = BF16 if os.environ.get("DIFF_TAB_BF16", "1") == "1" else FP32
TAB_NP = np.dtype("bfloat16") if TAB_DT is BF16 else np.dtype("float32")


# ================================================================ host prep
def _prep(feat, ef, W, b, src, dst):
    """Build per-core tiled edge metadata. Returns (in_maps, ntiles)."""
    feat = np.ascontiguousarray(np.asarray(feat), dtype=np.float32).reshape(N, ROW)
    ef = np.asarray(ef, dtype=np.float32)
    src = np.asarray(src, dtype=np.int64)
    dst = np.asarray(dst, dtype=np.int64)

    order = np.argsort(dst, kind="stable")
    s_src = src[order]
    s_dst = dst[order]
    s_ef = ef[order]

    core_edge_bounds = np.searchsorted(s_dst, np.arange(CORES + 1) * SPC)

    # ---- per-core greedy tiling (capacity per source half)
    per_core = []
    for c in range(CORES):
        lo, hi = core_edge_bounds[c], core_edge_bounds[c + 1]
        cs, cd, ce = s_src[lo:hi], s_dst[lo:hi] - c * SPC, s_ef[lo:hi]
        fwd_mask = cs < N
        counts_f = np.bincount(cd[fwd_mask], minlength=SPC)
        counts_b = np.bincount(cd[~fwd_mask], minlength=SPC)
        starts = np.concatenate([[0], np.cumsum(counts_f + counts_b)])
        tiles = []  # (slot_lo, slot_hi, edge_lo, edge_hi)
        slot = 0
        while slot < SPC:
            t_lo = slot
            nf = nb = 0
            while (
                slot < SPC
                and slot - t_lo < 128
                and nf + counts_f[slot] <= CAP
                and nb + counts_b[slot] <= CAP
            ):
                nf += counts_f[slot]
                nb += counts_b[slot]
                slot += 1
            if slot == t_lo:
                raise ValueError("node degree exceeds tile capacity")
            tiles.append((t_lo, slot, starts[t_lo], starts[slot]))
        per_core.append((cs, cd, ce, tiles))

    ntiles = max(len(p[3]) for p in per_core)
    TPC = ntiles * 128  # padded table rows per core

    # slot (local, per core) -> padded table position (local, per core)
    pos_local = np.zeros((CORES, SPC), np.int64)
    for c in range(CORES):
        tiles = per_core[c][3]
        for t, (sl, sh, _, _) in enumerate(tiles):
            pos_local[c, sl:sh] = t * 128 + np.arange(sh - sl)

    def padded_pos(glob):
        """global slot id in [0, 2N) -> row in padded [8*TPC] table."""
        cc = glob // SPC
        return cc * TPC + pos_local[cc, glob % SPC]

    # block-diagonal final weights: for block bi, half h: [128, 128] with
    # rows p = tq*32+f, cols m = (tq'-2h)*64+o, val = W[bi*32+f, o] iff tq==tq'
    Wf = np.asarray(W, np.float32)
    wblk = np.zeros((128, 14, 128), np.float32)
    for bi in range(7):
        for h in range(2):
            for tq in (2 * h, 2 * h + 1):
                wblk[
                    tq * 32 : (tq + 1) * 32,
                    bi * 2 + h,
                    (tq - 2 * h) * 64 : (tq - 2 * h + 1) * 64,
                ] = Wf[bi * 32 : (bi + 1) * 32, :]
    bias2 = np.tile(np.asarray(b, np.float32), 2).reshape(128, 1)

    def wrap16(vals):
        """int64 idx array (multiple of 16) -> [128, len//16] int16 wrapped."""
        return np.tile(vals.astype(np.int16).reshape(-1, 16).T, (8, 1))

    in_maps = []
    for c in range(CORES):
        cs, cd, ce, tiles = per_core[c]
        gidx0 = np.full((128, ntiles, 2, CAP // 16), -1, np.int16)
        gidx12 = np.full((128, ntiles, 2, CAP // 16), -1, np.int16)
        # null tiles: 16 valid row-0 reads per half, rest skipped (-1)
        gidx0[:, :, :, 0] = 0
        gidx0[:, :, :, HCA // 16] = 0
        gidx12[:, :, :, 0] = 0
        gidx12[:, :, :, HCA // 16] = 0
        cnts = np.full((ntiles, 2, 2), 16, np.int32)
        smat = np.zeros((ntiles, 128, 2 * CPF, 128), np.float32)
        for t, (sl, sh, el, eh) in enumerate(tiles):
            e_src = cs[el:eh]
            e_slot = cd[el:eh] - sl
            e_w = ce[el:eh]
            for side in (0, 1):
                m = (e_src < N) if side == 0 else (e_src >= N)
                srcs = e_src[m]
                s_slot = e_slot[m]
                s_w = e_w[m]
                # k=0 table: feat rows (src mod N); k>=1: padded table pos
                real0 = (srcs - side * N).astype(np.int64)
                real12 = padded_pos(srcs) - side * 4 * TPC
                n = len(real0)
                # exact runtime counts per 512-half: [real | <=16 zeros | -1s]
                # (-1 tail is skipped by the gather ucode; the runtime count
                # must equal the number of non-negative indices)
                mask = -np.ones(CAP, np.int64)  # -1 = skip, 0 = valid row 0
                mask[:n] = 0
                n0 = min(n, HCA)
                keep0 = min(16, HCA - n0)
                mask[n0 : n0 + keep0] = 0
                n1 = n - n0
                keep1 = min(16, HCB - n1)
                mask[HCA + n1 : HCA + n1 + keep1] = 0
                cnts[t, side, 0] = n0 + keep0
                cnts[t, side, 1] = n1 + keep1
                idx0 = np.where(mask < 0, -1, 0)
                idx12 = idx0.copy()
                idx0[:n] = real0
                idx12[:n] = real12
                s_slot = np.concatenate([s_slot, np.zeros(CAP - n, np.int64)])
                s_w = np.concatenate([s_w, np.zeros(CAP - n, np.float32)])
                # edge g -> (partition g%128, chunk g//128); idx wrapped by 16
                gidx0[:, t, side, :] = wrap16(idx0)
                gidx12[:, t, side, :] = wrap16(idx12)
                # S[p, cc, slot] = 2*ef for chunk cc = side*CPF + g//128
                # (Chebyshev 2*A folded in; k=0 output rescaled by 0.5)
                e_p = np.arange(CAP) % 128
                e_cc = side * CPF + np.arange(CAP) // 128
                smat[t, e_p, e_cc, s_slot] += 2.0 * s_w

        # x0 slice in padded-tile layout (xp source for step k=1)
        x0pad = np.zeros((TPC, ROW), np.float32)
        x0pad[pos_local[c]] = feat[(c * SPC + np.arange(SPC)) % N]

        # final-linear gather indices, per (block, 1024-node macro-chunk)
        if c < 4:
            nbase = c * SPC
        else:
            nbase = (c - 4) * SPC + NFIN
        fidx = np.zeros((128, 7, NF4, 1024 // 16), np.int16)
        fcnt = np.zeros(NF4, np.int32)
        for ch in range(NF4):
            nreal = min(1024, NFIN - ch * 1024)
            fkeep = min(16, 1024 - nreal)
            fcnt[ch] = nreal + fkeep
            nl = ch * 1024 + np.arange(1024)
            ng = nbase + np.minimum(nl, NFIN - 1)
            cfw = ng // SPC          # core owning fwd half (0..3)
            loc = ng - (c % 4) * SPC
            i_fwd = cfw * TPC + pos_local[cfw, ng % SPC]       # x12 fwd pos
            i_bwd = cfw * TPC + pos_local[cfw + 4, ng % SPC]   # x12 bwd (-4TPC)
            i_3f = pos_local[c % 4, loc]                 # x3 fwd (group slot 0)
            i_3b = pos_local[c % 4 + 4, loc]             # x3 bwd (group slot 1)
            # blocks: feat, x1f, x1b, x2f, x2b, x3f, x3b
            colmap = [ng, i_fwd, i_bwd, i_fwd, i_bwd, i_3f, i_3b]
            for bi in range(7):
                vals = np.asarray(colmap[bi], np.int64).copy()
                vals[nreal : nreal + fkeep] = 0
                vals[nreal + fkeep :] = -1
                fidx[:, bi, ch, :] = wrap16(vals)
        in_maps.append(
            {
                "feat": feat.astype(TAB_NP, copy=True),
                "gidx0": gidx0.reshape(128, ntiles * 2 * (CAP // 16)).copy(),
                "gidx12": gidx12.reshape(128, ntiles * 2 * (CAP // 16)).copy(),
                "smat": smat.reshape(ntiles * 128, 2 * CPF * 128).astype(TAB_NP),
                "x0pad": x0pad.astype(TAB_NP),
                "fidx": fidx.reshape(128, 7 * NF4 * 64).copy(),
                "gcnt": np.concatenate(
                    [cnts.reshape(-1), fcnt]
                ).reshape(1, ntiles * 4 + NF4),
                "wblk": wblk.reshape(128, 14 * 128).astype(np.dtype("bfloat16")),
                "bias2": bias2.copy(),
                "ident": np.eye(128, dtype=np.dtype("bfloat16")),
            }
        )
    return in_maps, ntiles


# ================================================================ bass program
def _build(ntiles):
    nc = bacc.Bacc(
        "TRN2",
        target_bir_lowering=False,
        debug=False,
        num_devices=CORES,
        num_swdge_queues=NQ,
    )

    TPC = ntiles * 128

    feat_t = nc.dram_tensor("feat", [N, ROW], TAB_DT, kind="ExternalInput")
    gidx0_t = nc.dram_tensor(
        "gidx0", [128, ntiles * 2 * (CAP // 16)], I16, kind="ExternalInput"
    )
    gidx12_t = nc.dram_tensor(
        "gidx12", [128, ntiles * 2 * (CAP // 16)], I16, kind="ExternalInput"
    )
    smat_t = nc.dram_tensor(
        "smat", [ntiles * 128, 2 * CPF * 128], TAB_DT, kind="ExternalInput"
    )
    x0pad_t = nc.dram_tensor("x0pad", [TPC, ROW], TAB_DT, kind="ExternalInput")
    fidx_t = nc.dram_tensor(
        "fidx", [128, 7 * NF4 * 64], I16, kind="ExternalInput"
    )
    wblk_t = nc.dram_tensor("wblk", [128, 14 * 128], BF16, kind="ExternalInput")
    bias2_t = nc.dram_tensor("bias2", [128, 1], FP32, kind="ExternalInput")
    ident_t = nc.dram_tensor("ident", [128, 128], BF16, kind="ExternalInput")
    gcnt_t = nc.dram_tensor(
        "gcnt", [1, ntiles * 4 + NF4], I32, kind="ExternalInput"
    )

    outT = nc.dram_tensor("outT", [T, OUTF, NFIN], FP32, kind="ExternalOutput")

    # internal DRAM: per-step slice tensors (padded tile rows)
    slice_bufs = [
        nc.dram_tensor(f"slice{k}", [TPC, ROW], TAB_DT) for k in range(STEPS)
    ]
    x1t = nc.dram_tensor("x1t", [CORES * TPC, ROW], TAB_DT, addr_space="Shared")
    x2t = nc.dram_tensor("x2t", [CORES * TPC, ROW], TAB_DT, addr_space="Shared")
    x3t = nc.dram_tensor("x3t", [2 * TPC, ROW], TAB_DT)

    rg_all = [list(range(CORES))]
    rg_pair = [[c, c + 4] for c in range(4)]

    warm_in = nc.dram_tensor("warm_in", [16, ROW], TAB_DT)
    warm_out = nc.dram_tensor("warm_out", [128, ROW], TAB_DT, addr_space="Shared")
    warm_out2 = nc.dram_tensor("warm_out2", [32, ROW], TAB_DT)

    ICW = CAP // 16  # idx columns per (tile, half)

    with tile.TileContext(nc, num_cores=CORES) as tc:
        with (
            tc.tile_pool(name="const", bufs=1) as constp,
            tc.tile_pool(name="meta", bufs=1) as metap,
        ):
            # step-0 prerequisites first so its gathers start ASAP; the
            # final-phase constants load behind them on the sync queue
            gcnt_s = metap.tile([1, ntiles * 4 + NF4], I32)
            nc.sync.dma_start(gcnt_s[:], gcnt_t[:])
            gidx0_s = metap.tile([128, ntiles * 2 * ICW], I16)
            nc.sync.dma_start(gidx0_s[:], gidx0_t[:])
            gidx12_s = metap.tile([128, ntiles * 2 * ICW], I16)
            nc.sync.dma_start(gidx12_s[:], gidx12_t[:])
            wblk_s = constp.tile([128, 14 * 128], BF16)
            nc.sync.dma_start(wblk_s[:], wblk_t[:])
            bias2_s = constp.tile([128, 1], FP32)
            nc.sync.dma_start(bias2_s[:], bias2_t[:])
            ident_s = constp.tile([128, 128], BF16)
            nc.sync.dma_start(ident_s[:], ident_t[:])
            fidx_s = metap.tile([128, 7 * NF4 * 64], I16)
            nc.sync.dma_start(fidx_s[:], fidx_t[:])
            cregs = [
                nc.alloc_register(mybir.EngineType.Pool, f"gcnt{i}")
                for i in range(4)
            ]

            # ---------------- diffusion steps
            with (
                tc.tile_pool(name="big", bufs=1) as bigp,
                tc.tile_pool(name="gat", bufs=3) as gatp,
                tc.tile_pool(name="sml", bufs=3) as smlp,
                tc.tile_pool(name="ys", bufs=3) as ysp,
                tc.tile_pool(name="xpp", bufs=3) as xpp,
                tc.tile_pool(name="ps", bufs=3, space="PSUM") as psp,
                tc.tile_pool(name="fin", bufs=2) as finp,
                tc.tile_pool(name="fps", bufs=1, space="PSUM") as fpsp,
            ):
                # tiny warmup collectives: pay CC setup/rendezvous under step 0
                nc.gpsimd.collective_compute(
                    "AllGather",
                    mybir.AluOpType.bypass,
                    replica_groups=rg_all,
                    ins=[warm_in[:].opt()],
                    outs=[warm_out[:].opt()],
                )
                nc.gpsimd.collective_compute(
                    "AllGather",
                    mybir.AluOpType.bypass,
                    replica_groups=rg_pair,
                    ins=[warm_in[:].opt()],
                    outs=[warm_out2[:].opt()],
                )

                # final-linear gather source blocks (x3 halves appended below)
                blocks = [
                    feat_t[0:N, :],
                    x1t[0 : 4 * TPC, :],
                    x1t[4 * TPC : 8 * TPC, :],
                    x2t[0 : 4 * TPC, :],
                    x2t[4 * TPC : 8 * TPC, :],
                    x3t[0:TPC, :],
                    x3t[TPC : 2 * TPC, :],
                ]
                pre_gf = {}

                y0_all = bigp.tile([128, ntiles * ROW], TAB_DT, name="y0")
                for k in range(STEPS):
                    if k == 0:
                        halves = [feat_t[0:N, :], feat_t[0:N, :]]
                        gsrc = gidx0_s
                    elif k == 1:
                        halves = [x1t[0 : 4 * TPC, :], x1t[4 * TPC : 8 * TPC, :]]
                        gsrc = gidx12_s
                    else:
                        halves = [x2t[0 : 4 * TPC, :], x2t[4 * TPC : 8 * TPC, :]]
                        gsrc = gidx12_s
                    for t in range(ntiles):
                        psum = psp.tile([128, ROW], FP32, tag="acc")
                        Ssb = smlp.tile([128, 2 * CPF * 128], TAB_DT, tag="S")
                        nc.sync.dma_start(Ssb[:], smat_t[t * 128 : (t + 1) * 128, :])
                        if k == 1:
                            xp = xpp.tile([128, ROW], TAB_DT, tag="xp")
                            nc.sync.dma_start(
                                xp[:], x0pad_t[t * 128 : (t + 1) * 128, :]
                            )
                        for side in (0, 1):
                            G = gatp.tile(
                                [128, CPF * ROW], TAB_DT, tag=f"G{side}",
                                name=f"G{side}",
                            )
                            if k == 0 and t < 3:
                                # first use of each rotating buffer: clear
                                # uninitialized SBUF (skipped gather rows are
                                # multiplied by S=0; 0*NaN would poison psum)
                                nc.vector.memset(G[:], 0.0)
                            icb = (t * 2 + side) * ICW
                            hcA = CPF - CPF // 2  # chunks in first half-gather
                            for hh in range(2):
                                ri = side * 2 + hh
                                nc.gpsimd.reg_load(
                                    cregs[ri],
                                    gcnt_s[0:1, (t * 2 + side) * 2 + hh :
                                           (t * 2 + side) * 2 + hh + 1],
                                )
                                glo = 0 if hh == 0 else hcA * ROW
                                ghi = hcA * ROW if hh == 0 else CPF * ROW
                                ilo = 0 if hh == 0 else HCA // 16
                                ihi = HCA // 16 if hh == 0 else ICW
                                nc.gpsimd.dma_gather(
                                    G[:, glo:ghi].rearrange(
                                        "p (c r) -> p c r",
                                        c=(hcA if hh == 0 else CPF - hcA),
                                    ),
                                    halves[side],
                                    gsrc[:, icb + ilo : icb + ihi],
                                    HCA if hh == 0 else HCB,
                                    cregs[ri],
                                    ROW,
                                    queue_num=side * 2 + hh,
                                )
                            for j in range(CPF):
                                cc = side * CPF + j
                                nc.tensor.matmul(
                                    out=psum[:],
                                    lhsT=Ssb[:, cc * 128 : (cc + 1) * 128],
                                    rhs=G[:, j * ROW : (j + 1) * ROW],
                                    start=(side == 0 and j == 0),
                                    stop=(side == 1 and j == CPF - 1),
                                )
                        # psum holds 2*A@x ; Chebyshev combine in ONE DVE op
                        if k == 0:
                            y_sl = y0_all[:, t * ROW : (t + 1) * ROW]
                            nc.vector.tensor_scalar(
                                out=y_sl,
                                in0=psum[:],
                                scalar1=0.5,
                                scalar2=None,
                                op0=mybir.AluOpType.mult,
                            )
                        else:
                            y_t = ysp.tile([128, ROW], TAB_DT, tag="y")
                            y_sl = y_t[:]
                            xp_sl = (
                                xp[:]
                                if k == 1
                                else y0_all[:, t * ROW : (t + 1) * ROW]
                            )
                            nc.vector.tensor_tensor(
                                out=y_sl,
                                in0=psum[:],
                                in1=xp_sl,
                                op=mybir.AluOpType.subtract,
                            )
                        # plain write into this tile's padded rows (Sync HWDGE)
                        nc.sync.dma_start(
                            slice_bufs[k][t * 128 : (t + 1) * 128, :],
                            y_sl,
                        )
                    if k == STEPS - 1:
                        # pre-emit x3-independent final gathers BEFORE the x3
                        # collective trigger: the trigger's rendezvous wait
                        # (pair-core skew) otherwise stalls the GPSIMD queue
                        for ch4 in range(2):
                            for bi in range(5):
                                Gf = finp.tile(
                                    [128, 8 * ROW], TAB_DT, tag=f"Gf{bi}",
                                    name=f"Gf{bi}",
                                )
                                pre_gf[(ch4, bi)] = Gf
                                ri = (ch4 * 5 + bi) % NQ
                                nc.gpsimd.reg_load(
                                    cregs[ri],
                                    gcnt_s[
                                        0:1, ntiles * 4 + ch4 : ntiles * 4 + ch4 + 1
                                    ],
                                )
                                nc.gpsimd.dma_gather(
                                    Gf[:].rearrange("p (c r) -> p c r", c=8),
                                    blocks[bi],
                                    fidx_s[
                                        :,
                                        (bi * NF4 + ch4) * 64 : (bi * NF4 + ch4 + 1)
                                        * 64,
                                    ],
                                    1024,
                                    cregs[ri],
                                    ROW,
                                    queue_num=ri,
                                )
                    nc.gpsimd.collective_compute(
                        "AllGather",
                        mybir.AluOpType.bypass,
                        replica_groups=rg_all if k < STEPS - 1 else rg_pair,
                        ins=[slice_bufs[k][:].opt()],
                        outs=[((x1t, x2t, x3t)[k])[:].opt()],
                    )

                # ---------------- final linear: node-major 1024-row gathers,
                # TensorE transpose to feature-major, block-diagonal weight
                # matmuls (no narrow regroup copies)
                for ch4 in range(NF4):
                    Gfs = []
                    for bi, tab in enumerate(blocks):
                        if (ch4, bi) in pre_gf:
                            Gfs.append(pre_gf[(ch4, bi)])
                            continue
                        Gf = finp.tile(
                            [128, 8 * ROW], TAB_DT, tag=f"Gf{bi}", name=f"Gf{bi}"
                        )
                        Gfs.append(Gf)
                        ri = (ch4 * 7 + bi) % NQ
                        nc.gpsimd.reg_load(
                            cregs[ri],
                            gcnt_s[0:1, ntiles * 4 + ch4 : ntiles * 4 + ch4 + 1],
                        )
                        nc.gpsimd.dma_gather(
                            Gf[:].rearrange("p (c r) -> p c r", c=8),
                            tab,
                            fidx_s[:, (bi * NF4 + ch4) * 64 : (bi * NF4 + ch4 + 1) * 64],
                            1024,
                            cregs[ri],
                            ROW,
                            queue_num=ri,
                        )
                    for half4 in range(2):  # 512-node halves of the macro-chunk
                        p0 = ch4 * 1024 + half4 * 512
                        if p0 >= NFIN:
                            break
                        ncols = min(512, NFIN - p0)
                        for hh in range(2):  # (t,f) granule: t in [4hh, 4hh+4)
                            hT = []
                            for bi in range(7):
                                hTb = finp.tile(
                                    [128, 512], BF16, tag=f"hT{bi}", name=f"hT{bi}"
                                )
                                hT.append(hTb[:])
                                for s4 in range(4):
                                    if p0 + s4 * 128 >= NFIN:
                                        break
                                    sub = half4 * 4 + s4
                                    ptr = fpsp.tile(
                                        [128, 128], BF16, tag=f"ptr{bi % 2}",
                                        name="ptr",
                                    )
                                    nc.tensor.transpose(
                                        out=ptr[:],
                                        in_=Gfs[bi][
                                            :, sub * ROW + hh * 128 : sub * ROW
                                            + (hh + 1) * 128
                                        ],
                                        identity=ident_s[:],
                                    )
                                    nc.vector.tensor_copy(
                                        hTb[:, s4 * 128 : (s4 + 1) * 128], ptr[:]
                                    )
                            for h2 in range(2):  # t pair within granule
                                pso = fpsp.tile(
                                    [128, 512], FP32, tag=f"pso{h2}", name="pso"
                                )
                                for bi in range(7):
                                    nc.tensor.matmul(
                                        out=pso[:],
                                        lhsT=wblk_s[
                                            :,
                                            (bi * 2 + h2) * 128 : (bi * 2 + h2 + 1)
                                            * 128,
                                        ],
                                        rhs=hT[bi],
                                        start=(bi == 0),
                                        stop=(bi == 6),
                                    )
                                stage = finp.tile(
                                    [128, 512], FP32, tag=f"st{h2}", name="st"
                                )
                                nc.scalar.activation(
                                    out=stage[:],
                                    in_=pso[:],
                                    func=mybir.ActivationFunctionType.Identity,
                                    bias=bias2_s[:],
                                    scale=1.0,
                                )
                                # rows m = (t - (4hh+2h2))*64 + o -> outT[t, o]
                                st = stage[:]
                                out_ap = bass.AP(
                                    outT,
                                    ((hh * 4 + 2 * h2) * OUTF) * NFIN + p0,
                                    [[NFIN, 128], [1, ncols]],
                                )
                                st_ap = bass.AP(
                                    st.tensor, st.offset, [st.ap[0], [1, ncols]]
                                )
                                nc.sync.dma_start(out=out_ap, in_=st_ap)

    nc.compile()
    return nc


_NC_CACHE = {}


def _get_nc(ntiles):
    if ntiles not in _NC_CACHE:
        _NC_CACHE[ntiles] = _build(ntiles)
    return _NC_CACHE[ntiles]


# ================================================================ entry point
def kernel(feat, ef, W, b, src, dst):
    from concourse.bass_utils import run_bass_kernel_spmd

    in_maps, ntiles = _prep(feat, ef, W, b, src, dst)
    nc = _get_nc(ntiles)
    res = run_bass_kernel_spmd(nc, in_maps, core_ids=list(range(CORES)))
    out = np.zeros((N, T, OUTF), np.float32)
    for c in range(CORES):
        o = np.asarray(res.results[c]["outT"], np.float32).reshape(T, OUTF, NFIN)
        if c < 4:
            nbase = c * SPC
        else:
            nbase = (c - 4) * SPC + NFIN
        out[nbase : nbase + NFIN] = o.transpose(2, 0, 1)
    return out
